# revision 9
# baseline (speedup 1.0000x reference)
"""Trainium2 Bass kernel for CapsNet dynamic routing (ClassCapsules).

Reference computation (B=256, R=1152, C=10, O=16, I=8, 3 routing iters):
    u_hat[b,r,c,o] = sum_i W[r,c,o,i] * x[b,r,i]
    b_ij = 0
    for it in 3:
        c_ij = softmax(b_ij, axis=1)                      # over c
        s = sum_r c_ij[r,c] * u_hat[b,r,c,o] + bias       # [B,C,O]
        v = squash(s)
        if it < 2:
            b_ij += mean_b sum_o u_hat[b,r,c,o] v[b,c,o]  # [R,C]
    return v[..., None]

u_hat ([B,R,C,O] = 189MB fp32) is never materialized.  Both routing
contractions are re-associated through the factorization
    s[b,co]    = x~[b,(ri)] @ (c∘W~)[(ri),(co)]
    agree[r,c] = sum_{i,o} W~[(ri),(co)] * G[(ri),(co)],
                 G = (1/B) x~^T v
with x~ = x viewed as [B, R*I] and W~ = W viewed as [R*I, C*O].

Distribution: R is sharded 8 ways (144 r's per core) for iterations 1-2.
Iteration 0's c is UNIFORM (softmax of zeros), so s0 = 0.1*(x~ @ W~)
does not depend on c at all: every core redundantly computes the full
s0 from replicated fp16 copies of x~/W~ — the loads and the 144-matmul
accumulation hide inside the cross-core launch-skew window that the
first collective would have to absorb anyway.  This removes iteration
0's AllReduce (and its +-30us of barrier-serialized latency) from the
critical path.  Iteration 1 sums the partial s with one fp16 AllReduce
(which doubles as the rank-sync barrier); iteration 2 uses a fp16
ReduceScatter and each core squashes + outputs its own batch rows.
agree/b_ij/c_ij are local to each core's r-shard.

All matmul operands are fp16 (host-precast); accumulation fp32 in PSUM.
Scalar-engine activation tables (Sqrt/Exp) are prefetched with dummy
ops so table loads stay off the critical path.
"""

import os
import sys
import types

sys.path.insert(0, "/opt/trn_rl_repo")

# Shim antenv.axon_hooks (absent on this image) so BASS_TRACE=1 profiling
# works through run_bass_kernel_spmd's axon path.  Harmless when unused.
try:
    import antenv.axon_hooks  # noqa: F401
except ImportError:
    try:
        _hooks = types.ModuleType("antenv.axon_hooks")
        _hooks._hook = None
        _hooks.set_axon_ntff_profile_hook = lambda h: setattr(_hooks, "_hook", h)
        _hooks.get_axon_ntff_profile_hook = lambda: _hooks._hook
        sys.modules["antenv.axon_hooks"] = _hooks
        import antenv
        antenv.axon_hooks = _hooks
        from trn_agent_boot.trn_boot import _ntff_profile_via_ctypes
        _hooks.set_axon_ntff_profile_hook(
            _ntff_profile_via_ctypes("/opt/axon/libaxon_pjrt.so")
        )
    except Exception:
        pass

import numpy as np

import concourse.bacc as bacc
import concourse.bass as bass
import concourse.tile as tile
from concourse import mybir
import concourse.bass_utils as _bass_utils
from concourse.bass_utils import run_bass_kernel_spmd

if os.environ.get("BASS_TRACE"):
    _bass_utils.upload_artifacts = lambda tmpdir: ""  # no bucket access here

LAST_RESULT = None

F32 = mybir.dt.float32
F16 = mybir.dt.float16
ALU = mybir.AluOpType
ACT = mybir.ActivationFunctionType

B, R, C, O, I = 256, 1152, 10, 16, 8
CO = C * O                      # 160
N_CORES = 8
R_LOC = R // N_CORES            # 144
RI_LOC = R_LOC * I              # 1152
NG = RI_LOC // 128              # 9 groups of 128 (r,i) rows
NB = B // 128                   # 2 batch partition chunks
P_SHARD = 128 // N_CORES        # 16 partition rows per core in ReduceScatter
ITERS = 3
RPG = 128 // I                  # 16 r's per group

WARM_MM = int(os.environ.get("K_WARM_MM", "24"))


def _squash(nc, eps_sb, t, n_part, nb, pool, out_dt=F16):
    """v = t * n2/((1+n2)*sqrt(n2+eps)); t: [n_part, nb, CO], reduce over o."""
    nc_ = nb * C
    tf = t.rearrange("p nb co -> p (nb co)")
    sq = pool.tile([n_part, nb * CO], F32, tag="sq")
    nc.vector.tensor_mul(sq, tf, tf)
    n2 = pool.tile([n_part, nc_], F32, tag="n2")
    nc.vector.reduce_sum(
        n2, sq.rearrange("p (nb c o) -> p nb c o", nb=nb, c=C),
        axis=mybir.AxisListType.X,
    )
    rt = pool.tile([n_part, nc_], F32, tag="rt")
    nc.scalar.activation(rt, n2, ACT.Sqrt, bias=eps_sb[:n_part])
    den = pool.tile([n_part, nc_], F32, tag="den")
    nc.vector.scalar_tensor_tensor(
        out=den, in0=n2, scalar=1.0, in1=rt, op0=ALU.add, op1=ALU.mult,
    )
    rec = pool.tile([n_part, nc_], F32, tag="rec")
    nc.vector.reciprocal(rec, den)
    fac = pool.tile([n_part, nc_], F32, tag="fac")
    nc.vector.tensor_mul(fac, n2, rec)
    v = pool.tile([n_part, nb, CO], out_dt, tag="v")
    fac_b = fac.rearrange(
        "p (nb c one) -> p nb c one", nb=nb, c=C
    ).broadcast_to([n_part, nb, C, O])
    nc.vector.tensor_tensor(
        out=v.rearrange("p nb (c o) -> p nb c o", c=C),
        in0=t.rearrange("p nb (c o) -> p nb c o", c=C),
        in1=fac_b,
        op=ALU.mult,
    )
    return v


def build():
    nc = bacc.Bacc("TRN2", target_bir_lowering=False, debug=False,
                   num_devices=N_CORES)

    xtf_d = nc.dram_tensor("xtf", [R * I, B], F16, kind="ExternalInput")
    wgf_d = nc.dram_tensor("wgf", [R * I, CO], F16, kind="ExternalInput")
    xt_d = nc.dram_tensor("xt", [RI_LOC, B], F16, kind="ExternalInput")
    xb_d = nc.dram_tensor("xb", [B, RI_LOC], F16, kind="ExternalInput")
    wg_d = nc.dram_tensor("wg", [RI_LOC, CO], F16, kind="ExternalInput")
    bias_d = nc.dram_tensor("biasf", [CO], F32, kind="ExternalInput")
    sel_d = nc.dram_tensor("sel", [128, RPG], F32, kind="ExternalInput")
    selT_d = nc.dram_tensor("selT", [RPG, 128], F32, kind="ExternalInput")
    y_d = nc.dram_tensor("y", [P_SHARD, NB * CO], F32, kind="ExternalOutput")

    rg = [list(range(N_CORES))]

    with tile.TileContext(nc) as tc:
        with (
            tc.tile_pool(name="singles", bufs=1) as singles,
            tc.tile_pool(name="work", bufs=2) as work,
            tc.tile_pool(name="small", bufs=3) as small,
            tc.tile_pool(name="psum_s", bufs=1, space="PSUM") as psum_s,
            tc.tile_pool(name="psum_g", bufs=2, space="PSUM") as psum_g,
            tc.tile_pool(name="dram", bufs=2, space="DRAM") as dram,
        ):
            cw_pool = work
            psum_misc = psum_s
            # ---- load inputs: replicated full x~/W~ first (s0 consumes
            # them chunk by chunk), then the per-core shard tensors.
            # Posts are spread over four engine queues — descriptor
            # posting serializes at ~1us per DMA within one queue.
            qs = [nc.sync, nc.scalar, nc.gpsimd]
            XTF, WGF = [], []
            for cc in range(N_CORES):
                tx = singles.tile([128, NG, B], F16, tag=f"xtf{cc}",
                                  name=f"xtf_sb{cc}")
                qs[(2 * cc) % 3].dma_start(
                    out=tx,
                    in_=xtf_d[cc * RI_LOC:(cc + 1) * RI_LOC, :].rearrange(
                        "(g p) b -> p g b", p=128),
                )
                XTF.append(tx)
                tw = singles.tile([128, NG, CO], F16, tag=f"wgf{cc}",
                                  name=f"wgf_sb{cc}")
                qs[(2 * cc + 1) % 3].dma_start(
                    out=tw,
                    in_=wgf_d[cc * RI_LOC:(cc + 1) * RI_LOC, :].rearrange(
                        "(g p) n -> p g n", p=128),
                )
                WGF.append(tw)

            XT = singles.tile([128, NG, B], F16)       # local x~ [(ri),b]
            nc.sync.dma_start(
                out=XT, in_=xt_d.ap().rearrange("(g p) b -> p g b", p=128)
            )
            XB = []                                    # x [b,(ri)] 2 p-chunks
            for kb in range(NB):
                t = singles.tile([128, RI_LOC], F16, tag=f"xb{kb}",
                                 name=f"xb_sb{kb}")
                qs[kb + 1].dma_start(out=t, in_=xb_d[kb * 128:(kb + 1) * 128, :])
                XB.append(t)
            WG = singles.tile([128, NG, CO], F16)      # local W~ [(ri),(co)]
            nc.gpsimd.dma_start(
                out=WG, in_=wg_d.ap().rearrange("(g p) n -> p g n", p=128)
            )
            biasb = singles.tile([128, CO], F32)
            nc.scalar.dma_start(
                out=biasb,
                in_=bass.AP(tensor=bias_d, offset=0, ap=[[0, 128], [1, CO]]),
            )
            sel_sb = singles.tile([128, RPG], F32)
            nc.sync.dma_start(out=sel_sb, in_=sel_d[:, :])
            selT_sb = singles.tile([RPG, 128], F32)
            nc.sync.dma_start(out=selT_sb, in_=selT_d[:, :])

            eps_sb = singles.tile([128, 1], F32)
            nc.vector.memset(eps_sb, 1e-8)
            junk = singles.tile([1, 1], F32)
            nc.vector.memset(junk, 1.0)
            # Prefetch the Sqrt activation table while inputs load.
            tl0 = singles.tile([1, 1], F32, tag="tl0")
            nc.scalar.activation(tl0, junk, ACT.Sqrt)

            esr = None   # [16, 99]: exp(b_ij) ++ 1/sum_c exp(b_ij)

            for it in range(ITERS):
                if it == 0:
                    # ---- s0 = 0.1*(x~full @ W~full): c is uniform, so
                    # every core computes the full [256,160] locally.
                    s_ps = [psum_s.tile([128, CO], F32, tag=f"s{kb}",
                                        name=f"s_ps{kb}_0")
                            for kb in range(NB)]
                    for cc in range(N_CORES):
                        for g in range(NG):
                            for kb in range(NB):
                                nc.tensor.matmul(
                                    s_ps[kb],
                                    XTF[cc][:, g, kb * 128:(kb + 1) * 128],
                                    WGF[cc][:, g, :],
                                    start=(cc == 0 and g == 0),
                                    stop=(cc == N_CORES - 1 and g == NG - 1),
                                )
                    t = work.tile([128, NB, CO], F32, tag="t", name="t_0")
                    for kb in range(NB):
                        nc.vector.scalar_tensor_tensor(
                            out=t[:, kb, :], in0=s_ps[kb], scalar=0.1,
                            in1=biasb, op0=ALU.mult, op1=ALU.add,
                        )
                else:
                    # ---- CW = c∘W~ from esr of the previous iteration ----
                    cp_ps = psum_misc.tile([128, NG * C + NG], F32, tag="cp",
                                           name=f"cp_ps_{it}")
                    nc.tensor.matmul(cp_ps, selT_sb, esr, start=True, stop=True)
                    cp_sb = small.tile([128, NG * C + NG], F32, tag="cpart",
                                       name=f"cp_sb_{it}")
                    nc.vector.tensor_copy(cp_sb, cp_ps)
                    CW = cw_pool.tile([128, NG, CO], F16, tag="cw",
                                      name=f"cw_{it}")
                    NGP = 4          # groups on DVE via stt; rest on gpsimd
                    cn = small.tile([128, (NG - NGP) * C], F32, tag="cn",
                                    name=f"cn_{it}")
                    rec_b3 = cp_sb[:, NG * C + NGP:NG * C + NG].rearrange(
                        "p (g one) -> p g one", one=1
                    ).broadcast_to([128, NG - NGP, C])
                    nc.vector.tensor_tensor(
                        out=cn.rearrange("p (g c) -> p g c", g=NG - NGP),
                        in0=cp_sb[:, NGP * C:NG * C].rearrange(
                            "p (g c) -> p g c", g=NG - NGP),
                        in1=rec_b3, op=ALU.mult,
                    )
                    for g in range(NG):
                        if g < NGP:
                            e_b = cp_sb[:, g * C:(g + 1) * C].rearrange(
                                "p (c one) -> p c one", one=1
                            ).broadcast_to([128, C, O])
                            nc.vector.scalar_tensor_tensor(
                                out=CW[:, g, :].rearrange(
                                    "p (c o) -> p c o", c=C),
                                in0=WG[:, g, :].rearrange(
                                    "p (c o) -> p c o", c=C),
                                scalar=cp_sb[:, NG * C + g:NG * C + g + 1],
                                in1=e_b,
                                op0=ALU.mult, op1=ALU.mult,
                            )
                        else:
                            c_b = cn[:, (g - NGP) * C:(g - NGP + 1) * C
                                     ].rearrange(
                                "p (c one) -> p c one", one=1
                            ).broadcast_to([128, C, O])
                            nc.gpsimd.tensor_tensor(
                                out=CW[:, g, :].rearrange(
                                    "p (c o) -> p c o", c=C),
                                in0=WG[:, g, :].rearrange(
                                    "p (c o) -> p c o", c=C),
                                in1=c_b, op=ALU.mult,
                            )

                    # ---- s partial: [256,160] = x~^T @ CW, K = (ri) ----
                    s_ps = [psum_s.tile([128, CO], F32, tag=f"s{kb}",
                                        name=f"s_ps{kb}_{it}")
                            for kb in range(NB)]
                    for kb in range(NB):
                        for g in range(NG):
                            nc.tensor.matmul(
                                s_ps[kb],
                                XT[:, g, kb * 128:(kb + 1) * 128],
                                CW[:, g, :],
                                start=(g == 0),
                                stop=(g == NG - 1),
                            )

                    # Partition-major collective buffer: row p holds s for
                    # batches (p, 128+p) at columns [0:CO] / [CO:2*CO].
                    cc_in = dram.tile([128, NB * CO], F16, tag="cc_in",
                                      name=f"cc_in_{it}")
                    for kb in range(NB):
                        s_stage = work.tile([128, CO], F16, tag=f"sstage{kb}",
                                            name=f"s_stage{kb}_{it}")
                        nc.vector.tensor_copy(s_stage, s_ps[kb])
                        nc.sync.dma_start(
                            out=cc_in[:, kb * CO:(kb + 1) * CO], in_=s_stage
                        )

                    if it < ITERS - 1:
                        # ---- AllReduce s (doubles as the rank barrier) ----
                        cc_out = dram.tile([128, NB * CO], F16, tag="cc_out",
                                           name=f"cc_out_{it}")
                        nc.gpsimd.collective_compute(
                            "AllReduce", ALU.add, replica_groups=rg,
                            ins=[cc_in.opt()], outs=[cc_out.opt()],
                        )
                        s_sb = work.tile([128, NB, CO], F16, tag="ssb",
                                         name=f"s_sb_{it}")
                        nc.sync.dma_start(
                            out=s_sb.rearrange("p nb co -> p (nb co)"),
                            in_=cc_out[:, :],
                        )
                        # Keep the PE HAM busy during the AllReduce so the
                        # G matmuls start at full clock.
                        warm_ps = psum_misc.tile([128, 512], F32,
                                                 tag="warmps",
                                                 name=f"warm_ps_{it}")
                        warm_rhs = XT[:, 0, :]        # [128, 256] static
                        warm_lhs = XT[:, 0, :128]     # [128, 128] fp16
                        for wi in range(WARM_MM):
                            nc.tensor.matmul(
                                warm_ps[:, :B], warm_lhs, warm_rhs,
                                start=(wi == 0), stop=True,
                                skip_group_check=True,
                            )
                        t = work.tile([128, NB, CO], F32, tag="t",
                                      name=f"t_{it}")
                        bias_b = biasb.rearrange(
                            "p (one co) -> p one co", one=1
                        ).broadcast_to([128, NB, CO])
                        nc.vector.scalar_tensor_tensor(
                            out=t, in0=s_sb, scalar=1.0,
                            in1=bias_b, op0=ALU.mult, op1=ALU.add,
                        )
                    else:
                        # ---- final iter: ReduceScatter; own shard only ----
                        # Shard k of the flat [128*NB*CO] buffer = partition
                        # rows [16k, 16k+16) = batches 16k+j and 128+16k+j.
                        rs_out = dram.tile([P_SHARD * NB * CO], F16,
                                           tag="rs_out")
                        nc.gpsimd.collective_compute(
                            "ReduceScatter", ALU.add, replica_groups=rg,
                            ins=[cc_in.opt()], outs=[rs_out[:]],
                        )
                        s_sb = work.tile([P_SHARD, NB, CO], F16, tag="fs")
                        nc.sync.dma_start(
                            out=s_sb,
                            in_=rs_out.rearrange("(p nb n) -> p nb n",
                                                 n=CO, nb=NB),
                        )
                        t = work.tile([P_SHARD, NB, CO], F32, tag="ft")
                        bias_b1 = biasb[:P_SHARD, :].rearrange(
                            "p (one co) -> p one co", one=1
                        ).broadcast_to([P_SHARD, NB, CO])
                        nc.vector.scalar_tensor_tensor(
                            out=t, in0=s_sb, scalar=1.0,
                            in1=bias_b1, op0=ALU.mult, op1=ALU.add,
                        )
                        v = _squash(nc, eps_sb, t, P_SHARD, NB, work,
                                    out_dt=F32)
                        nc.sync.dma_start(
                            out=y_d[:, :],
                            in_=v.rearrange("p nb co -> p (nb co)"),
                        )
                        break

                v_sb = _squash(nc, eps_sb, t, 128, NB, work, out_dt=F16)
                # Prefetch the Exp table (runs during the G matmuls).
                tlE = small.tile([1, 1], F32, tag="tlE", name=f"tlE_{it}")
                nc.scalar.activation(tlE, junk, ACT.Exp)

                # ---- G = (1/B) x~^T v ; agree = sum_io W∘G ----
                Q_all = small.tile([128, NG * C], F32, tag="qall",
                                   name=f"qall_{it}")
                p9 = work.tile([128, NG, CO], F16, tag="p9",
                               name=f"p9_{it}")
                for g in range(NG):
                    g_ps = psum_g.tile([128, CO], F32, tag="gps",
                                       name=f"g_ps_{it}_{g}")
                    for kb in range(NB):
                        nc.tensor.matmul(
                            g_ps,
                            XB[kb][:, g * 128:(g + 1) * 128],
                            v_sb[:, kb, :],
                            start=(kb == 0),
                            stop=(kb == NB - 1),
                        )
                    nc.vector.scalar_tensor_tensor(
                        out=p9[:, g, :], in0=g_ps, scalar=1.0 / B,
                        in1=WG[:, g, :], op0=ALU.mult, op1=ALU.mult,
                    )
                    if g == 3 or g == 7 or g == 8:
                        lo = 0 if g == 3 else (4 if g == 7 else 8)
                        nc.vector.reduce_sum(
                            Q_all[:, lo * C:(g + 1) * C],
                            p9[:, lo:g + 1, :].rearrange(
                                "p g (c o) -> p (g c) o", c=C),
                            axis=mybir.AxisListType.X,
                        )
                agree_ps = psum_misc.tile([RPG, NG * C], F32, tag="agree",
                                          name=f"agree_{it}")
                nc.tensor.matmul(agree_ps, sel_sb, Q_all,
                                 start=True, stop=True)

                # ---- exp(b_ij) updated multiplicatively:
                # exp(b_prev + agree) = exp(b_prev) * exp(agree) ----
                esr_prev = esr
                esr = small.tile([RPG, NG * C + NG], F32, tag="esr",
                                 name=f"esr_{it}")
                if it == 0:
                    nc.scalar.activation(esr[:, :NG * C], agree_ps, ACT.Exp)
                else:
                    eexp = small.tile([RPG, NG * C], F32, tag="eexp",
                                      name=f"eexp_{it}")
                    nc.scalar.activation(eexp, agree_ps, ACT.Exp)
                    nc.vector.tensor_mul(
                        esr[:, :NG * C], esr_prev[:, :NG * C], eexp
                    )
                # Prefetch Sqrt for the next squash (runs during CW/s).
                tlS = small.tile([1, 1], F32, tag="tlS", name=f"tlS_{it}")
                nc.scalar.activation(tlS, junk, ACT.Sqrt)
                den = small.tile([RPG, NG], F32, tag="sden",
                                 name=f"den_{it}")
                nc.vector.reduce_sum(
                    den,
                    esr[:, :NG * C].rearrange("p (g c) -> p g c", g=NG),
                    axis=mybir.AxisListType.X,
                )
                nc.vector.reciprocal(esr[:, NG * C:], den)

    nc.compile()
    return nc


_NC = None


def kernel(x: np.ndarray, W: np.ndarray, bias: np.ndarray) -> np.ndarray:
    global _NC
    if _NC is None:
        _NC = build()

    x = np.ascontiguousarray(x, dtype=np.float32)
    W = np.ascontiguousarray(W, dtype=np.float32)
    bias = np.ascontiguousarray(bias, dtype=np.float32)

    biasf = bias.reshape(CO)
    sel = np.zeros((128, RPG), dtype=np.float32)
    sel[np.arange(128), np.arange(128) // I] = 1.0
    selT = np.ascontiguousarray(sel.T)

    xf = x.reshape(B, R * I)                              # [B,(r,i)]
    xtf = np.ascontiguousarray(xf.T).astype(np.float16)   # [(r,i),B]
    wgf = np.ascontiguousarray(
        W.transpose(0, 3, 1, 2).reshape(R * I, CO)).astype(np.float16)

    in_maps = []
    for k in range(N_CORES):
        r0, r1 = k * R_LOC, (k + 1) * R_LOC
        xk = x[:, r0:r1, :].reshape(B, RI_LOC)          # [B,(r,i)]
        wk = W[r0:r1].transpose(0, 3, 1, 2).reshape(RI_LOC, CO)  # [(ri),(co)]
        in_maps.append({
            "xtf": xtf,
            "wgf": wgf,
            "xt": np.ascontiguousarray(xk.T).astype(np.float16),
            "xb": np.ascontiguousarray(xk).astype(np.float16),
            "wg": np.ascontiguousarray(wk).astype(np.float16),
            "biasf": biasf,
            "sel": sel,
            "selT": selT,
        })

    global LAST_RESULT
    res = run_bass_kernel_spmd(
        _NC, in_maps, list(range(N_CORES)),
        trace=bool(os.environ.get("BASS_TRACE")),
    )
    LAST_RESULT = res
    # Reassemble: core k, row j, chunk kb  ->  batch kb*128 + 16*k + j.
    out = np.empty((B, CO), dtype=np.float32)
    for k in range(N_CORES):
        yk = res.results[k]["y"].reshape(P_SHARD, NB, CO)
        for kb in range(NB):
            out[kb * 128 + P_SHARD * k: kb * 128 + P_SHARD * (k + 1)] = \
                yk[:, kb, :]
    return out.reshape(B, C, O)[..., None].astype(np.float32)


# revision 10
# speedup vs baseline: 1.2085x; 1.2085x over previous
"""Trainium2 Bass kernel for CapsNet dynamic routing (ClassCapsules).

Reference computation (B=256, R=1152, C=10, O=16, I=8, 3 routing iters):
    u_hat[b,r,c,o] = sum_i W[r,c,o,i] * x[b,r,i]
    b_ij = 0
    for it in 3:
        c_ij = softmax(b_ij, axis=1)                      # over c
        s = sum_r c_ij[r,c] * u_hat[b,r,c,o] + bias       # [B,C,O]
        v = squash(s)
        if it < 2:
            b_ij += mean_b sum_o u_hat[b,r,c,o] v[b,c,o]  # [R,C]
    return v[..., None]

u_hat ([B,R,C,O] = 189MB fp32) is never materialized.  Both routing
contractions are re-associated through the factorization
    s[b,co]    = x~[b,(ri)] @ (c∘W~)[(ri),(co)]
    agree[r,c] = sum_{i,o} W~[(ri),(co)] * G[(ri),(co)],
                 G = (1/B) x~^T v
with x~ = x viewed as [B, R*I] and W~ = W viewed as [R*I, C*O].

Distribution: R is sharded 8 ways (144 r's per core) for iterations 1-2.
Iteration 0's c is UNIFORM (softmax of zeros), so s0 = 0.1*(x~ @ W~)
does not depend on c at all: every core redundantly computes the full
s0 from replicated fp16 copies of x~/W~ — the loads and the 144-matmul
accumulation hide inside the cross-core launch-skew window that the
first collective would have to absorb anyway.  This removes iteration
0's AllReduce (and its +-30us of barrier-serialized latency) from the
critical path.  Iteration 1 sums the partial s with one fp16 AllReduce
(which doubles as the rank-sync barrier); iteration 2 uses a fp16
ReduceScatter and each core squashes + outputs its own batch rows.
agree/b_ij/c_ij are local to each core's r-shard.

All matmul operands are fp16 (host-precast); accumulation fp32 in PSUM.
Scalar-engine activation tables (Sqrt/Exp) are prefetched with dummy
ops so table loads stay off the critical path.
"""

import os
import sys
import types

sys.path.insert(0, "/opt/trn_rl_repo")

# Shim antenv.axon_hooks (absent on this image) so BASS_TRACE=1 profiling
# works through run_bass_kernel_spmd's axon path.  Harmless when unused.
try:
    import antenv.axon_hooks  # noqa: F401
except ImportError:
    try:
        _hooks = types.ModuleType("antenv.axon_hooks")
        _hooks._hook = None
        _hooks.set_axon_ntff_profile_hook = lambda h: setattr(_hooks, "_hook", h)
        _hooks.get_axon_ntff_profile_hook = lambda: _hooks._hook
        sys.modules["antenv.axon_hooks"] = _hooks
        import antenv
        antenv.axon_hooks = _hooks
        from trn_agent_boot.trn_boot import _ntff_profile_via_ctypes
        _hooks.set_axon_ntff_profile_hook(
            _ntff_profile_via_ctypes("/opt/axon/libaxon_pjrt.so")
        )
    except Exception:
        pass

import numpy as np

import concourse.bacc as bacc
import concourse.bass as bass
import concourse.tile as tile
from concourse import mybir
import concourse.bass_utils as _bass_utils
from concourse.bass_utils import run_bass_kernel_spmd

if os.environ.get("BASS_TRACE"):
    _bass_utils.upload_artifacts = lambda tmpdir: ""  # no bucket access here

LAST_RESULT = None

F32 = mybir.dt.float32
F16 = mybir.dt.float16
ALU = mybir.AluOpType
ACT = mybir.ActivationFunctionType

B, R, C, O, I = 256, 1152, 10, 16, 8
CO = C * O                      # 160
N_CORES = 8
R_LOC = R // N_CORES            # 144
RI_LOC = R_LOC * I              # 1152
NG = RI_LOC // 128              # 9 groups of 128 (r,i) rows
NB = B // 128                   # 2 batch partition chunks
P_SHARD = 128 // N_CORES        # 16 partition rows per core in ReduceScatter
ITERS = 3
RPG = 128 // I                  # 16 r's per group

WARM_MM = int(os.environ.get("K_WARM_MM", "24"))


def _squash(nc, eps_sb, t, n_part, nb, pool, out_dt=F16):
    """v = t * n2/((1+n2)*sqrt(n2+eps)); t: [n_part, nb, CO], reduce over o."""
    nc_ = nb * C
    tf = t.rearrange("p nb co -> p (nb co)")
    sq = pool.tile([n_part, nb * CO], F32, tag="sq")
    nc.vector.tensor_mul(sq, tf, tf)
    n2 = pool.tile([n_part, nc_], F32, tag="n2")
    nc.vector.reduce_sum(
        n2, sq.rearrange("p (nb c o) -> p nb c o", nb=nb, c=C),
        axis=mybir.AxisListType.X,
    )
    rt = pool.tile([n_part, nc_], F32, tag="rt")
    nc.scalar.activation(rt, n2, ACT.Sqrt, bias=eps_sb[:n_part])
    den = pool.tile([n_part, nc_], F32, tag="den")
    nc.vector.scalar_tensor_tensor(
        out=den, in0=n2, scalar=1.0, in1=rt, op0=ALU.add, op1=ALU.mult,
    )
    rec = pool.tile([n_part, nc_], F32, tag="rec")
    nc.vector.reciprocal(rec, den)
    fac = pool.tile([n_part, nc_], F32, tag="fac")
    nc.vector.tensor_mul(fac, n2, rec)
    v = pool.tile([n_part, nb, CO], out_dt, tag="v")
    fac_b = fac.rearrange(
        "p (nb c one) -> p nb c one", nb=nb, c=C
    ).broadcast_to([n_part, nb, C, O])
    nc.vector.tensor_tensor(
        out=v.rearrange("p nb (c o) -> p nb c o", c=C),
        in0=t.rearrange("p nb (c o) -> p nb c o", c=C),
        in1=fac_b,
        op=ALU.mult,
    )
    return v


def build():
    nc = bacc.Bacc("TRN2", target_bir_lowering=False, debug=False,
                   num_devices=N_CORES)

    # Host-packed partition-major inputs: one contiguous DMA per chunk
    # (strided loads cost ~2.6us of descriptor posting each).
    # pka: [128, 8, 3744] = per r-shard cc: xtf (9*256) ++ wgf (9*160).
    # pkb: [128, 3744] = local xt (9*256) ++ local wg (9*160).
    # pkc: [128, 2, 1152] = local x batch-major, 2 partition chunks.
    PKA = NG * B + NG * CO          # 3744
    pka_d = [nc.dram_tensor(f"pka{j}", [128, 2 * PKA], F16,
                            kind="ExternalInput") for j in range(4)]
    pkb_d = nc.dram_tensor("pkb", [128, PKA], F16, kind="ExternalInput")
    pkc_d = nc.dram_tensor("pkc", [128, NB * RI_LOC], F16,
                           kind="ExternalInput")
    bias_d = nc.dram_tensor("biasf", [CO], F32, kind="ExternalInput")
    sel_d = nc.dram_tensor("sel", [128, RPG], F32, kind="ExternalInput")
    selT_d = nc.dram_tensor("selT", [RPG, 128], F32, kind="ExternalInput")
    y_d = nc.dram_tensor("y", [P_SHARD, NB * CO], F32, kind="ExternalOutput")

    rg = [list(range(N_CORES))]

    with tile.TileContext(nc) as tc:
        with (
            tc.tile_pool(name="singles", bufs=1) as singles,
            tc.tile_pool(name="work", bufs=2) as work,
            tc.tile_pool(name="small", bufs=3) as small,
            tc.tile_pool(name="psum_s", bufs=1, space="PSUM") as psum_s,
            tc.tile_pool(name="psum_g", bufs=2, space="PSUM") as psum_g,
            tc.tile_pool(name="dram", bufs=2, space="DRAM") as dram,
        ):
            cw_pool = work
            psum_misc = psum_s
            # ---- load inputs: 4 packed chunks carry the replicated full
            # x~/W~ (s0 consumes them pair by pair), then the local packs.
            PKA = NG * B + NG * CO
            A = []
            for j in range(4):
                ta = singles.tile([128, 2, PKA], F16, tag=f"pka{j}",
                                  name=f"pka_sb{j}")
                nc.sync.dma_start(
                    out=ta, in_=pka_d[j].ap().rearrange(
                        "p (two n) -> p two n", n=PKA))
                A.append(ta)
            tb = singles.tile([128, PKA], F16, name="pkb_sb")
            nc.sync.dma_start(out=tb, in_=pkb_d[:, :])
            tco = singles.tile([128, NB, RI_LOC], F16, name="pkc_sb")
            nc.sync.dma_start(
                out=tco, in_=pkc_d.ap().rearrange(
                    "p (nb n) -> p nb n", n=RI_LOC))
            XTF = [A[cc // 2][:, cc % 2, :NG * B].rearrange(
                       "p (g b) -> p g b", g=NG) for cc in range(N_CORES)]
            WGF = [A[cc // 2][:, cc % 2, NG * B:].rearrange(
                       "p (g n) -> p g n", g=NG) for cc in range(N_CORES)]
            XT = tb[:, :NG * B].rearrange("p (g b) -> p g b", g=NG)
            WG = tb[:, NG * B:].rearrange("p (g n) -> p g n", g=NG)
            XB = [tco[:, kb, :] for kb in range(NB)]
            biasb = singles.tile([128, CO], F32)
            nc.sync.dma_start(
                out=biasb,
                in_=bass.AP(tensor=bias_d, offset=0, ap=[[0, 128], [1, CO]]),
            )
            sel_sb = singles.tile([128, RPG], F32)
            nc.sync.dma_start(out=sel_sb, in_=sel_d[:, :])
            selT_sb = singles.tile([RPG, 128], F32)
            nc.sync.dma_start(out=selT_sb, in_=selT_d[:, :])

            eps_sb = singles.tile([128, 1], F32)
            nc.vector.memset(eps_sb, 1e-8)
            junk = singles.tile([1, 1], F32)
            nc.vector.memset(junk, 1.0)
            # Prefetch the Sqrt activation table while inputs load.
            tl0 = singles.tile([1, 1], F32, tag="tl0")
            nc.scalar.activation(tl0, junk, ACT.Sqrt)

            esr = None   # [16, 99]: exp(b_ij) ++ 1/sum_c exp(b_ij)

            for it in range(ITERS):
                if it == 0:
                    # ---- s0 = 0.1*(x~full @ W~full): c is uniform, so
                    # every core computes the full [256,160] locally.
                    s_ps = [psum_s.tile([128, CO], F32, tag=f"s{kb}",
                                        name=f"s_ps{kb}_0")
                            for kb in range(NB)]
                    for cc in range(N_CORES):
                        for g in range(NG):
                            for kb in range(NB):
                                nc.tensor.matmul(
                                    s_ps[kb],
                                    XTF[cc][:, g, kb * 128:(kb + 1) * 128],
                                    WGF[cc][:, g, :],
                                    start=(cc == 0 and g == 0),
                                    stop=(cc == N_CORES - 1 and g == NG - 1),
                                )
                    t = work.tile([128, NB, CO], F32, tag="t", name="t_0")
                    for kb in range(NB):
                        nc.vector.scalar_tensor_tensor(
                            out=t[:, kb, :], in0=s_ps[kb], scalar=0.1,
                            in1=biasb, op0=ALU.mult, op1=ALU.add,
                        )
                else:
                    # ---- CW = c∘W~ from esr of the previous iteration ----
                    cp_ps = psum_misc.tile([128, NG * C + NG], F32, tag="cp",
                                           name=f"cp_ps_{it}")
                    nc.tensor.matmul(cp_ps, selT_sb, esr, start=True, stop=True)
                    cp_sb = small.tile([128, NG * C + NG], F32, tag="cpart",
                                       name=f"cp_sb_{it}")
                    nc.vector.tensor_copy(cp_sb, cp_ps)
                    CW = cw_pool.tile([128, NG, CO], F16, tag="cw",
                                      name=f"cw_{it}")
                    NGP = 4          # groups on DVE via stt; rest on gpsimd
                    cn = small.tile([128, (NG - NGP) * C], F32, tag="cn",
                                    name=f"cn_{it}")
                    rec_b3 = cp_sb[:, NG * C + NGP:NG * C + NG].rearrange(
                        "p (g one) -> p g one", one=1
                    ).broadcast_to([128, NG - NGP, C])
                    nc.vector.tensor_tensor(
                        out=cn.rearrange("p (g c) -> p g c", g=NG - NGP),
                        in0=cp_sb[:, NGP * C:NG * C].rearrange(
                            "p (g c) -> p g c", g=NG - NGP),
                        in1=rec_b3, op=ALU.mult,
                    )
                    for g in range(NG):
                        if g < NGP:
                            e_b = cp_sb[:, g * C:(g + 1) * C].rearrange(
                                "p (c one) -> p c one", one=1
                            ).broadcast_to([128, C, O])
                            nc.vector.scalar_tensor_tensor(
                                out=CW[:, g, :].rearrange(
                                    "p (c o) -> p c o", c=C),
                                in0=WG[:, g, :].rearrange(
                                    "p (c o) -> p c o", c=C),
                                scalar=cp_sb[:, NG * C + g:NG * C + g + 1],
                                in1=e_b,
                                op0=ALU.mult, op1=ALU.mult,
                            )
                        else:
                            c_b = cn[:, (g - NGP) * C:(g - NGP + 1) * C
                                     ].rearrange(
                                "p (c one) -> p c one", one=1
                            ).broadcast_to([128, C, O])
                            nc.gpsimd.tensor_tensor(
                                out=CW[:, g, :].rearrange(
                                    "p (c o) -> p c o", c=C),
                                in0=WG[:, g, :].rearrange(
                                    "p (c o) -> p c o", c=C),
                                in1=c_b, op=ALU.mult,
                            )

                    # ---- s partial: [256,160] = x~^T @ CW, K = (ri) ----
                    s_ps = [psum_s.tile([128, CO], F32, tag=f"s{kb}",
                                        name=f"s_ps{kb}_{it}")
                            for kb in range(NB)]
                    for kb in range(NB):
                        for g in range(NG):
                            nc.tensor.matmul(
                                s_ps[kb],
                                XT[:, g, kb * 128:(kb + 1) * 128],
                                CW[:, g, :],
                                start=(g == 0),
                                stop=(g == NG - 1),
                            )

                    # Partition-major collective buffer: row p holds s for
                    # batches (p, 128+p) at columns [0:CO] / [CO:2*CO].
                    cc_in = dram.tile([128, NB * CO], F16, tag="cc_in",
                                      name=f"cc_in_{it}")
                    for kb in range(NB):
                        s_stage = work.tile([128, CO], F16, tag=f"sstage{kb}",
                                            name=f"s_stage{kb}_{it}")
                        nc.vector.tensor_copy(s_stage, s_ps[kb])
                        nc.sync.dma_start(
                            out=cc_in[:, kb * CO:(kb + 1) * CO], in_=s_stage
                        )

                    if it < ITERS - 1:
                        # ---- AllReduce s (doubles as the rank barrier) ----
                        cc_out = dram.tile([128, NB * CO], F16, tag="cc_out",
                                           name=f"cc_out_{it}")
                        nc.gpsimd.collective_compute(
                            "AllReduce", ALU.add, replica_groups=rg,
                            ins=[cc_in.opt()], outs=[cc_out.opt()],
                        )
                        s_sb = work.tile([128, NB, CO], F16, tag="ssb",
                                         name=f"s_sb_{it}")
                        nc.sync.dma_start(
                            out=s_sb.rearrange("p nb co -> p (nb co)"),
                            in_=cc_out[:, :],
                        )
                        # Keep the PE HAM busy during the AllReduce so the
                        # G matmuls start at full clock.
                        warm_ps = psum_misc.tile([128, 512], F32,
                                                 tag="warmps",
                                                 name=f"warm_ps_{it}")
                        warm_rhs = XT[:, 0, :]        # [128, 256] static
                        warm_lhs = XT[:, 0, :128]     # [128, 128] fp16
                        for wi in range(WARM_MM):
                            nc.tensor.matmul(
                                warm_ps[:, :B], warm_lhs, warm_rhs,
                                start=(wi == 0), stop=True,
                                skip_group_check=True,
                            )
                        t = work.tile([128, NB, CO], F32, tag="t",
                                      name=f"t_{it}")
                        bias_b = biasb.rearrange(
                            "p (one co) -> p one co", one=1
                        ).broadcast_to([128, NB, CO])
                        nc.vector.scalar_tensor_tensor(
                            out=t, in0=s_sb, scalar=1.0,
                            in1=bias_b, op0=ALU.mult, op1=ALU.add,
                        )
                    else:
                        # ---- final iter: ReduceScatter; own shard only ----
                        # Shard k of the flat [128*NB*CO] buffer = partition
                        # rows [16k, 16k+16) = batches 16k+j and 128+16k+j.
                        rs_out = dram.tile([P_SHARD * NB * CO], F16,
                                           tag="rs_out")
                        nc.gpsimd.collective_compute(
                            "ReduceScatter", ALU.add, replica_groups=rg,
                            ins=[cc_in.opt()], outs=[rs_out[:]],
                        )
                        s_sb = work.tile([P_SHARD, NB, CO], F16, tag="fs")
                        nc.sync.dma_start(
                            out=s_sb,
                            in_=rs_out.rearrange("(p nb n) -> p nb n",
                                                 n=CO, nb=NB),
                        )
                        t = work.tile([P_SHARD, NB, CO], F32, tag="ft")
                        bias_b1 = biasb[:P_SHARD, :].rearrange(
                            "p (one co) -> p one co", one=1
                        ).broadcast_to([P_SHARD, NB, CO])
                        nc.vector.scalar_tensor_tensor(
                            out=t, in0=s_sb, scalar=1.0,
                            in1=bias_b1, op0=ALU.mult, op1=ALU.add,
                        )
                        v = _squash(nc, eps_sb, t, P_SHARD, NB, work,
                                    out_dt=F32)
                        nc.sync.dma_start(
                            out=y_d[:, :],
                            in_=v.rearrange("p nb co -> p (nb co)"),
                        )
                        break

                v_sb = _squash(nc, eps_sb, t, 128, NB, work, out_dt=F16)
                # Prefetch the Exp table (runs during the G matmuls).
                tlE = small.tile([1, 1], F32, tag="tlE", name=f"tlE_{it}")
                nc.scalar.activation(tlE, junk, ACT.Exp)

                # ---- G = (1/B) x~^T v ; agree = sum_io W∘G ----
                Q_all = small.tile([128, NG * C], F32, tag="qall",
                                   name=f"qall_{it}")
                p9 = work.tile([128, NG, CO], F16, tag="p9",
                               name=f"p9_{it}")
                for g in range(NG):
                    g_ps = psum_g.tile([128, CO], F32, tag="gps",
                                       name=f"g_ps_{it}_{g}")
                    for kb in range(NB):
                        nc.tensor.matmul(
                            g_ps,
                            XB[kb][:, g * 128:(g + 1) * 128],
                            v_sb[:, kb, :],
                            start=(kb == 0),
                            stop=(kb == NB - 1),
                        )
                    nc.vector.scalar_tensor_tensor(
                        out=p9[:, g, :], in0=g_ps, scalar=1.0 / B,
                        in1=WG[:, g, :], op0=ALU.mult, op1=ALU.mult,
                    )
                    if g == 3 or g == 7 or g == 8:
                        lo = 0 if g == 3 else (4 if g == 7 else 8)
                        nc.vector.reduce_sum(
                            Q_all[:, lo * C:(g + 1) * C],
                            p9[:, lo:g + 1, :].rearrange(
                                "p g (c o) -> p (g c) o", c=C),
                            axis=mybir.AxisListType.X,
                        )
                agree_ps = psum_misc.tile([RPG, NG * C], F32, tag="agree",
                                          name=f"agree_{it}")
                nc.tensor.matmul(agree_ps, sel_sb, Q_all,
                                 start=True, stop=True)

                # ---- exp(b_ij) updated multiplicatively:
                # exp(b_prev + agree) = exp(b_prev) * exp(agree) ----
                esr_prev = esr
                esr = small.tile([RPG, NG * C + NG], F32, tag="esr",
                                 name=f"esr_{it}")
                if it == 0:
                    nc.scalar.activation(esr[:, :NG * C], agree_ps, ACT.Exp)
                else:
                    eexp = small.tile([RPG, NG * C], F32, tag="eexp",
                                      name=f"eexp_{it}")
                    nc.scalar.activation(eexp, agree_ps, ACT.Exp)
                    nc.vector.tensor_mul(
                        esr[:, :NG * C], esr_prev[:, :NG * C], eexp
                    )
                # Prefetch Sqrt for the next squash (runs during CW/s).
                tlS = small.tile([1, 1], F32, tag="tlS", name=f"tlS_{it}")
                nc.scalar.activation(tlS, junk, ACT.Sqrt)
                den = small.tile([RPG, NG], F32, tag="sden",
                                 name=f"den_{it}")
                nc.vector.reduce_sum(
                    den,
                    esr[:, :NG * C].rearrange("p (g c) -> p g c", g=NG),
                    axis=mybir.AxisListType.X,
                )
                nc.vector.reciprocal(esr[:, NG * C:], den)

    nc.compile()
    return nc


_NC = None


def kernel(x: np.ndarray, W: np.ndarray, bias: np.ndarray) -> np.ndarray:
    global _NC
    if _NC is None:
        _NC = build()

    x = np.ascontiguousarray(x, dtype=np.float32)
    W = np.ascontiguousarray(W, dtype=np.float32)
    bias = np.ascontiguousarray(bias, dtype=np.float32)

    biasf = bias.reshape(CO)
    sel = np.zeros((128, RPG), dtype=np.float32)
    sel[np.arange(128), np.arange(128) // I] = 1.0
    selT = np.ascontiguousarray(sel.T)

    xf = x.reshape(B, R * I)                              # [B,(r,i)]
    xtf = np.ascontiguousarray(xf.T).astype(np.float16)   # [(r,i),B]
    wgf = np.ascontiguousarray(
        W.transpose(0, 3, 1, 2).reshape(R * I, CO)).astype(np.float16)
    # Partition-major packs: row p, shard cc -> xtf (9*256) ++ wgf (9*160).
    xtf_pm = xtf.reshape(N_CORES, NG, 128, B).transpose(2, 0, 1, 3)
    wgf_pm = wgf.reshape(N_CORES, NG, 128, CO).transpose(2, 0, 1, 3)
    pka = np.concatenate(
        [xtf_pm.reshape(128, N_CORES, NG * B),
         wgf_pm.reshape(128, N_CORES, NG * CO)], axis=2
    ).astype(np.float16)  # [128, 8, 3744]
    pkas = [np.ascontiguousarray(pka[:, 2 * j:2 * j + 2, :]).reshape(128, -1)
            for j in range(4)]

    in_maps = []
    for k in range(N_CORES):
        r0, r1 = k * R_LOC, (k + 1) * R_LOC
        xk = x[:, r0:r1, :].reshape(B, RI_LOC)          # [B,(r,i)]
        wk = W[r0:r1].transpose(0, 3, 1, 2).reshape(RI_LOC, CO)  # [(ri),(co)]
        xkt = np.ascontiguousarray(xk.T).astype(np.float16)   # [(ri),B]
        wkt = wk.astype(np.float16)
        pkb = np.concatenate(
            [xkt.reshape(NG, 128, B).transpose(1, 0, 2).reshape(128, NG * B),
             wkt.reshape(NG, 128, CO).transpose(1, 0, 2).reshape(128, NG * CO)],
            axis=1).astype(np.float16)                  # [128, 3744]
        pkc = np.ascontiguousarray(
            xk.astype(np.float16).reshape(NB, 128, RI_LOC)
            .transpose(1, 0, 2).reshape(128, NB * RI_LOC))
        m = {f"pka{j}": pkas[j] for j in range(4)}
        m.update({
            "pkb": np.ascontiguousarray(pkb),
            "pkc": pkc,
            "biasf": biasf,
            "sel": sel,
            "selT": selT,
        })
        in_maps.append(m)

    global LAST_RESULT
    res = run_bass_kernel_spmd(
        _NC, in_maps, list(range(N_CORES)),
        trace=bool(os.environ.get("BASS_TRACE")),
    )
    LAST_RESULT = res
    # Reassemble: core k, row j, chunk kb  ->  batch kb*128 + 16*k + j.
    out = np.empty((B, CO), dtype=np.float32)
    for k in range(N_CORES):
        yk = res.results[k]["y"].reshape(P_SHARD, NB, CO)
        for kb in range(NB):
            out[kb * 128 + P_SHARD * k: kb * 128 + P_SHARD * (k + 1)] = \
                yk[:, kb, :]
    return out.reshape(B, C, O)[..., None].astype(np.float32)
